# revision 36
# baseline (speedup 1.0000x reference)
"""Causal self-attention with RoPE on 8 Trainium2 NeuronCores.

Problem (hardcoded): x [2, 2048, 1024] f32, w_qkv [1024, 3072], w_out [1024, 1024],
16 heads x head_dim 64, RoPE base 10000, causal softmax, out = attn @ w_out.

Sharding: DP over batch (2) x TP over head-groups (4 heads/core) = 8 cores.
Each core computes QKV for its 4 heads, full causal attention, and a partial
output projection against its 256 rows of w_out. Host sums the 4 partials per
batch element.

Design (cost-model driven, all matmuls bf16 = 1 cyc/row):
  - RoPE via algebraic identity: roped = cosq + tan * rot(cosq), where
    cos/sin are half-symmetric so rot(cos*q) = cos*rot(q).  rot() is a +-1
    permutation matmul on PE (no shuffle DMAs).  The cos-multiply doubles as
    the PSUM->SBUF eviction of the projection.
  - All attention is q-major in 512-wide chunks.  P@V is flipped (P
    stationary, V [128,65] moving: 65 PE cols per q-tile instead of 512);
    each of the 4 per-q-tile accumulators owns a full PSUM bank because PSUM
    accumulation state is per bank.  Column 64 accumulates the softmax
    denominator from a ones-column in V, landing per-q-partition for a
    tensor-scalar normalize (no broadcast).
  - The other 4 PSUM banks form one manually-slotted ring tile shared by
    score strips, projection, rope and output-projection tiles.  Adjacent
    ring slots let the exp of two non-diagonal chunks fuse into one
    1024-wide activation op, halving ACT instruction overhead.
  - Head 0 fuses with phase 1 (exp starts ~7us in); Q/K projection for
    heads 2-3 interleaves into head 1.  Normalized attention stages in
    [q-part, chan] tiles; the two heads of a ct-pair fill the two 64-chan
    halves and one [128,128] DMA-crossbar transpose lands both in attnT.
  - Output projection per q-chunk interleaves right after head 3 normalizes
    that chunk; its PSUM->SBUF bounce moves to ACT in the tail (after the
    last exp) and the store DMA is split across both DMA queues.
"""
import numpy as np
import ml_dtypes

import concourse.bacc as bacc
import concourse.tile as tile
from concourse import mybir
from concourse.bass_utils import run_bass_kernel_spmd

F32 = mybir.dt.float32
BF16 = mybir.dt.bfloat16
EXP = mybir.ActivationFunctionType.Exp

NP_BF16 = ml_dtypes.bfloat16

B, S, D = 2, 2048, 1024
H, HD = 16, 64
HPC = 4              # heads per core
CV = HPC * HD        # 256 v channels per core
NKT = S // 128       # 16 k-tiles
NSC = S // 512       # 4 seq chunks
SCALE = 1.0 / np.sqrt(HD)
ROPE_BASE = 10000.0


def _build_nc():
    nc = bacc.Bacc(None, target_bir_lowering=False, debug=False)

    xb8 = nc.declare_dram_parameter("xb8", [4, 128, 8, 512], BF16, isOutput=False)
    wqkb = nc.declare_dram_parameter("wqkb", [128, 8, 512], BF16, isOutput=False)
    wvb = nc.declare_dram_parameter("wvb", [128, 8, 256], BF16, isOutput=False)
    wob = nc.declare_dram_parameter("wob", [128, 2, D], BF16, isOutput=False)
    cosb = nc.declare_dram_parameter("cosb", [128, S], BF16, isOutput=False)
    tanb = nc.declare_dram_parameter("tanb", [128, S], BF16, isOutput=False)
    permb = nc.declare_dram_parameter("permb", [128, 128], BF16, isOutput=False)
    utrib = nc.declare_dram_parameter("utrib", [128, 128], BF16, isOutput=False)
    out = nc.declare_dram_parameter("out", [S, D], F32, isOutput=True)

    with tile.TileContext(nc) as tc:
        with (
            tc.tile_pool(name="const", bufs=1) as const,
            tc.tile_pool(name="qkt", bufs=1) as qkt_pool,
            tc.tile_pool(name="vsb", bufs=1) as vsb_pool,
            tc.tile_pool(name="pt", bufs=6) as pt_pool,
            tc.tile_pool(name="rope", bufs=2) as rope_pool,
            tc.tile_pool(name="attn", bufs=1) as attn_pool,
            tc.tile_pool(name="nrm", bufs=3) as nrm,
            tc.tile_pool(name="outp", bufs=4) as outp,
            tc.tile_pool(name="ps", bufs=1, space="PSUM") as ps,
        ):
            # ---- constants / inputs ----
            cos_sb = const.tile([128, S], BF16, name="cos")
            tan_sb = const.tile([128, S], BF16, name="tan")
            perm_sb = const.tile([128, 128], BF16, name="perm")
            utri_sb = const.tile([128, 128], BF16, name="utri")
            wqk_sb = const.tile([128, 8, 512], BF16, name="wqk")
            wv_sb = const.tile([128, 8, 256], BF16, name="wv")
            wo_sb = const.tile([128, 2, D], BF16, name="wo")
            xp_sb = const.tile([128, 8, 4, 512], BF16, name="xp")

            # DMA queues: scalar (ACT) handles the tables + wqk before the
            # first exp arrives; x chunks split across sync/gpsimd by seq.
            nc.scalar.dma_start(out=perm_sb, in_=permb[:, :])
            # ct2 then ct0 weight slices first: the first two projections
            # only need those 128-column blocks
            nc.scalar.dma_start(out=wqk_sb[:, :, 256:384], in_=wqkb[:, :, 256:384])
            nc.scalar.dma_start(out=wqk_sb[:, :, 0:128], in_=wqkb[:, :, 0:128])
            nc.scalar.dma_start(out=cos_sb, in_=cosb[:, :])
            nc.scalar.dma_start(out=wqk_sb[:, :, 128:256], in_=wqkb[:, :, 128:256])
            nc.scalar.dma_start(out=wqk_sb[:, :, 384:512], in_=wqkb[:, :, 384:512])
            nc.scalar.dma_start(out=tan_sb, in_=tanb[:, :])
            nc.scalar.dma_start(out=utri_sb, in_=utrib[:, :])
            nc.gpsimd.dma_start(out=wv_sb, in_=wvb[:, :, :])
            for sc in range(4):
                eng = (nc.sync, nc.gpsimd)[sc % 2]
                eng.dma_start(out=xp_sb[:, :, sc, :], in_=xb8[sc, :, :, :])
            nc.sync.dma_start(out=wo_sb, in_=wob[:, :, :])

            # PSUM ring: [128,512] tiles, 4 bufs = 4 banks
            _ctr = [0]

            def slot():
                _ctr[0] += 1
                return ps.tile(
                    [128, 512], F32, tag="strip", bufs=4, name=f"sp{_ctr[0]}"
                )

            # PE p-state warmup: burn dummy matmuls on the first-arriving
            # table so the tensor engine is at full clock (3us continuous
            # busy) when real work lands.
            for i in range(20):
                nc.tensor.matmul(slot()[:, 0:128], perm_sb, perm_sb, start=True, stop=True)

            # persistent phase-1 outputs
            QKT = [qkt_pool.tile([128, S], BF16, name=f"qkt{t}") for t in range(4)]
            Vsb = [vsb_pool.tile([128, HPC, 65], BF16, name=f"v{k}") for k in range(NKT)]

            # attnT[qc]: [chan 128, ct 2, q 512], written via the crossbar
            # transposes, read by the output projection
            attnT = [
                attn_pool.tile([128, 2, 512], BF16, name=f"attnT{qc}")
                for qc in range(NSC)
            ]
            # normalized-attention staging in [q-part, chan] layout
            aqp = [attn_pool.tile([128, 128], BF16, name=f"aqp{qt}") for qt in range(NKT)]

            def qk_proj(ct, sc):
                """Project q/k channel-tile ct for seq chunk sc; fold cos."""
                sl = slice(sc * 512, (sc + 1) * 512)
                qp = slot()
                for d in range(8):
                    nc.tensor.matmul(
                        qp,
                        wqk_sb[:, d, ct * 128 : (ct + 1) * 128],
                        xp_sb[:, d, sc, :],
                        start=(d == 0),
                        stop=(d == 7),
                    )
                nc.vector.tensor_mul(QKT[ct][:, sl], qp, cos_sb[:, sl])

            def qk_rope(ct, sc):
                """QKT[ct] chunk sc: += tan * (Perm @ cosq)."""
                sl = slice(sc * 512, (sc + 1) * 512)
                rot = slot()
                nc.tensor.matmul(rot, perm_sb, QKT[ct][:, sl], start=True, stop=True)
                tmp = rope_pool.tile([128, 512], BF16, name="ropetmp")
                nc.vector.tensor_mul(tmp, rot, tan_sb[:, sl])
                nc.gpsimd.tensor_add(QKT[ct][:, sl], QKT[ct][:, sl], tmp)

            def v_group(st):
                """Project v for seq tile st (128 positions, all 4 heads)."""
                vp = slot()[:, 0:256].rearrange("p (h c) -> p h c", h=4)
                for d in range(8):
                    nc.tensor.matmul(
                        vp,
                        xp_sb[:, d, st // 4, (st % 4) * 128 : (st % 4) * 128 + 128],
                        wv_sb[:, d, :],
                        start=(d == 0),
                        stop=(d == 7),
                    )
                nc.vector.tensor_copy(Vsb[st][:, :, 0:64], vp)
                nc.vector.memset(Vsb[st][:, :, 64:65], 1.0)

            def pv(h, qc, kt, acc, p_t, lo):
                """Flipped P@V: p_t q-tile sub-chunks are stationary, V is
                moving; acc[i] col 64 collects the denominator."""
                for i in range(lo, 4):
                    nc.tensor.matmul(
                        acc[i][:, 0:65],
                        p_t[:, i * 128 : (i + 1) * 128],
                        Vsb[kt][:, h, :],
                        start=(kt == 0),
                        stop=(kt == qc * 4 + i),
                    )

            def scores_mm(h, qc, kt, dst, o0):
                hh = h % 2
                nc.tensor.matmul(
                    dst[:, o0:512],
                    QKT[2 + h // 2][hh * 64 : hh * 64 + 64, kt * 128 : (kt + 1) * 128],
                    QKT[h // 2][hh * 64 : hh * 64 + 64, qc * 512 + o0 : (qc + 1) * 512],
                    start=True,
                    stop=True,
                )

            def attn_pair(h, qc, kt, acc):
                """Two non-diagonal chunks (kt, kt+1)."""
                for k in (kt, kt + 1):
                    sps = slot()
                    scores_mm(h, qc, k, sps, 0)
                    p_t = pt_pool.tile([128, 512], BF16, name="p_t")
                    nc.scalar.activation(p_t, sps, EXP, scale=SCALE)
                    pv(h, qc, k, acc, p_t, 0)

            def attn_diag(h, qc, kt, acc):
                """Diagonal chunk: exact width from the k boundary + triangle
                mask on the diagonal block."""
                r = kt % 4
                o0 = r * 128
                sps = slot()
                scores_mm(h, qc, kt, sps, o0)
                p_t = pt_pool.tile([128, 512], BF16, name="p_t")
                nc.scalar.activation(p_t[:, o0:512], sps[:, o0:512], EXP, scale=SCALE)
                nc.vector.tensor_mul(
                    p_t[:, o0 : o0 + 128], p_t[:, o0 : o0 + 128], utri_sb
                )
                pv(h, qc, kt, acc, p_t, r)

            def normalize_i(h, qc, i, acc, tail=False):
                """Normalize q-tile qc*4+i the moment its accumulator closes:
                reciprocal of the denominator column, scale the 64 attn
                channels into the [q-part, chan] staging tile; once both
                heads of the ct-pair are in, one [128,128] crossbar transpose
                lands them in attnT.  tail: chain the output projection of
                the q-tile right behind its transpose."""
                hh = h % 2
                qt = qc * 4 + i
                rc = nrm.tile([128, 1], F32, name="rc")
                nc.vector.reciprocal(rc, acc[i][:, 64:65])
                nc.vector.tensor_scalar_mul(
                    aqp[qt][:, hh * 64 : hh * 64 + 64], acc[i][:, 0:64], rc
                )
                if hh == 1:
                    nc.sync.dma_start_transpose(
                        attnT[qc][:, h // 2, i * 128 : (i + 1) * 128],
                        aqp[qt],
                    )

            class Dripper:
                """Spread a list of emission thunks across chunk sites so
                PE-only work never bursts long enough to starve ACT."""

                def __init__(self, units, sites):
                    self.units = list(units)
                    self.stride = max(1, sites // max(len(self.units), 1))
                    self.n = 0

                def maybe(self):
                    self.n += 1
                    if self.units and self.n % self.stride == 0:
                        self.units.pop(0)()

                def flush(self):
                    for u in self.units:
                        u()
                    self.units = []

            def head_qc(h, qc, drip=None, with_v=False, tail=False):
                """One q-chunk of head h's q-major attention."""
                acc = [
                    ps.tile([128, 512], F32, tag="acc", bufs=4, name=f"o{h}_{qc}_{i}")
                    for i in range(4)
                ]
                for kt in range(0, qc * 4, 2):
                    if drip is not None:
                        drip.maybe()
                    attn_pair(h, qc, kt, acc)
                for kt in range(qc * 4, qc * 4 + 4):
                    if with_v:
                        v_group(kt)
                    if drip is not None:
                        drip.maybe()
                    attn_diag(h, qc, kt, acc)
                    normalize_i(h, qc, kt - qc * 4, acc)
                if tail:
                    for i in range(4):
                        ph3_st(qc, i)

            def ph3_st(qc, sti):
                """Output projection for one 128-row seq tile."""
                st = qc * 4 + sti
                sl = slice(sti * 128, (sti + 1) * 128)
                op = [slot(), slot()]
                for ec in range(2):
                    for ct in range(2):
                        nc.tensor.matmul(
                            op[ec],
                            attnT[qc][:, ct, sl],
                            wo_sb[:, ct, ec * 512 : (ec + 1) * 512],
                            start=(ct == 0),
                            stop=(ct == 1),
                        )
                ob = outp.tile([128, 2, 512], F32, name="ob")
                for ec in range(2):
                    # ACT shares copies only in the tail (after the last exp)
                    if qc == 3 and ec == 1:
                        nc.scalar.copy(ob[:, ec, :], op[ec])
                    else:
                        nc.vector.tensor_copy(ob[:, ec, :], op[ec])
                    eng = (nc.sync, nc.gpsimd)[ec]
                    eng.dma_start(
                        out=out[st * 128 : (st + 1) * 128, ec * 512 : (ec + 1) * 512],
                        in_=ob[:, ec, :],
                    )

            # ---- schedule ----
            # Two passes, qc-major over head pairs.  Phase-1 work for q-chunk
            # qc+1 (and the output projection of qc-1 in pass 2) drips one
            # unit at a time between the chunks of q-chunk qc, so the exp
            # stream never sees a long PE-only burst.
            qk_proj(2, 0)
            qk_proj(0, 0)
            qk_rope(2, 0)
            qk_rope(0, 0)
            for qc in range(NSC):
                units = []
                if qc < 3:
                    n = qc + 1
                    units += [
                        lambda n=n: qk_proj(2, n),
                        lambda n=n: qk_rope(2, n),
                        lambda n=n: qk_proj(0, n),
                        lambda n=n: qk_rope(0, n),
                    ]
                    units += [lambda st=st: v_group(st) for st in range(4 * n, 4 * n + 4)]
                else:
                    units += [
                        lambda: qk_proj(3, 0),
                        lambda: qk_rope(3, 0),
                        lambda: qk_proj(1, 0),
                        lambda: qk_rope(1, 0),
                    ]
                drip = Dripper(units, 2 * (2 * qc + 4))
                head_qc(0, qc, drip, with_v=(qc == 0))
                head_qc(1, qc, drip)
                drip.flush()
            for qc in range(NSC):
                units = []
                if qc < 3:
                    n = qc + 1
                    units += [
                        lambda n=n: qk_proj(3, n),
                        lambda n=n: qk_rope(3, n),
                        lambda n=n: qk_proj(1, n),
                        lambda n=n: qk_rope(1, n),
                    ]
                if qc > 0:
                    units += [lambda qc=qc, s=s: ph3_st(qc - 1, s) for s in range(4)]
                drip = Dripper(units, 2 * (2 * qc + 4))
                head_qc(2, qc, drip)
                head_qc(3, qc, drip)
                drip.flush()
            for s in range(4):
                ph3_st(3, s)
    nc.compile()
    return nc


def _host_tables():
    half = HD // 2
    inv_freq = 1.0 / (ROPE_BASE ** (np.arange(0, half, dtype=np.float64) / half))
    ang = np.arange(S, dtype=np.float64)[:, None] * inv_freq[None, :]  # [S, 32]
    cosT = np.cos(ang).T  # [32, S]
    sinT = np.sin(ang).T
    cos64 = np.concatenate([cosT, cosT], axis=0)  # [64, S]
    tan64 = np.concatenate([sinT / cosT, sinT / cosT], axis=0)
    cosb = np.tile(cos64, (2, 1))
    tanb = np.tile(tan64, (2, 1))

    # rot permutation (sign-folded): rot[c] = -x[c+32], rot[c+32] = x[c]
    # per 64-channel head block; PermT[r, c] so that rot = PermT.T @ x
    permT = np.zeros((128, 128), dtype=np.float64)
    for blk in range(2):
        o = blk * 64
        for c in range(32):
            permT[o + c + 32, o + c] = -1.0
            permT[o + c, o + c + 32] = 1.0

    kk = np.arange(128)[:, None]
    qq = np.arange(128)[None, :]
    utri = (qq >= kk).astype(np.float64)
    return (
        np.ascontiguousarray(cosb.astype(NP_BF16)),
        np.ascontiguousarray(tanb.astype(NP_BF16)),
        np.ascontiguousarray(permT.astype(NP_BF16)),
        np.ascontiguousarray(utri.astype(NP_BF16)),
    )


def _dtiles(w, d_in, width):
    """[d_in, width] -> [128, d_in//128, width] bf16 d-tiled."""
    t = w.reshape(d_in // 128, 128, width).transpose(1, 0, 2)
    return np.ascontiguousarray(t.astype(NP_BF16))


_NC_CACHE = None


def kernel(x, w_qkv, w_out):
    global _NC_CACHE
    x = np.asarray(x, dtype=np.float32)
    w_qkv = np.asarray(w_qkv, dtype=np.float32)
    w_out = np.asarray(w_out, dtype=np.float32)

    cosb, tanb, permb, utrib = _host_tables()
    wq = w_qkv[:, 0:D]
    wk = w_qkv[:, D : 2 * D]
    wv = w_qkv[:, 2 * D : 3 * D]

    in_maps = []
    for c in range(8):
        b, hg = c // 4, c % 4
        cols = slice(hg * CV, (hg + 1) * CV)
        xT = np.ascontiguousarray(x[b].T)  # [1024, 2048]
        # x d-tiled then split into per-sc chunks: [4, 128, 8, 512]
        xd = _dtiles(xT, D, S).reshape(128, 8, 4, 512).transpose(2, 0, 1, 3)
        wqk = np.concatenate([wq[:, cols], wk[:, cols]], axis=1)  # [1024, 512]
        wo = w_out[cols, :]  # [256, 1024]
        in_maps.append(
            {
                "xb8": np.ascontiguousarray(xd),
                "wqkb": _dtiles(wqk, D, 512),
                "wvb": _dtiles(wv[:, cols], D, 256),
                "wob": _dtiles(wo, 256, D),
                "cosb": cosb,
                "tanb": tanb,
                "permb": permb,
                "utrib": utrib,
            }
        )

    if _NC_CACHE is None:
        _NC_CACHE = _build_nc()
    res = run_bass_kernel_spmd(_NC_CACHE, in_maps, core_ids=list(range(8)))
    out = np.zeros((B, S, D), dtype=np.float32)
    for c in range(8):
        out[c // 4] += res.results[c]["out"]
    return out
